# revision 47
# baseline (speedup 1.0000x reference)
"""Multi-head graph attention (GAT-style) Trainium2 kernel, v7.

Problem: out[b,h,i,o] = softmax_j(mask(leakyrelu_0.2(src[b,h,i] + dst[b,h,j])))
         @ h_prime[b,h,:,:] + bias
with h_prime = h @ w[h], src/dst = tanh(h_prime) @ a_src/a_dst.

Pure data-parallel over the 512-graph batch across 8 NeuronCores (64
graphs/core).  The kernel SOFTWARE-PIPELINES pairs of graphs three deep: iteration k
emits stage1(pair k) [tanh / coef rows / h_prime], stage2a(pair k-2)
[prelu / exp / mask], stage1b(pair k-1) [coef gathers + logits matmuls],
stage2b(pair k-2) [numerator / output].  Engines have strict-FIFO
queues, so this emission order keeps ACT fed (tanh of a later pair runs
while an earlier pair's logits chain is in flight) and keeps a stalled
numerator from blocking logits matmuls in the PE queue.

PSUM (8 banks): one shared [128,512] pool (bufs=2) rotates h_primeT
halves, coef rows, h_prime, and numerator tiles; logits tiles are
[128,1024] (2 banks) x bufs=3 so the pipelined emission never inverts
writer/reader order on a bank.

The unnormalized numerator + row sums ship to HBM in bf16; the softmax
division, bias add, and final transpose happen on the host.
"""

import numpy as np

BS, N, HEADS, DIN, DOUT = 512, 128, 8, 64, 64
NCORES = 8
BSH = BS // NCORES  # graphs per core

_cache = {}


def _build_nc():
    import concourse.bass as bass
    import concourse.mybir as mybir
    import concourse.tile as tile

    f32 = mybir.dt.float32
    f16 = mybir.dt.float16
    bf16 = mybir.dt.bfloat16
    AF = mybir.ActivationFunctionType
    HW = DOUT + 1  # 65: per-head numerator cols + row-sum column

    nc = bass.Bass("TRN2", target_bir_lowering=False, debug=False)

    # inp[b] = [hTr | adjT] side by side: cols 0-127 hT (DIN rows doubled),
    # cols 128-255 adjT (0/1).
    inp_d = nc.dram_tensor("inp", [BSH, 128, 2 * N], bf16, kind="ExternalInput").ap()
    w_allr_d = nc.dram_tensor("w_allr", [128, HEADS * DOUT], bf16, kind="ExternalInput").ap()
    a_mats_d = nc.dram_tensor("a_mats", [128, 128], bf16, kind="ExternalInput").ap()
    blockind_d = nc.dram_tensor("blockind", [HEADS, HEADS * N], f16, kind="ExternalInput").ap()
    out_d = nc.dram_tensor("out", [BSH, N, HEADS * HW], bf16, kind="ExternalOutput").ap()

    with tile.TileContext(nc) as tc:
        with (
            tc.tile_pool(name="consts", bufs=1) as cpool,
            tc.tile_pool(name="inbuf", bufs=4) as inpool,
            tc.tile_pool(name="mid", bufs=4) as midpool,
            tc.tile_pool(name="hpabuf", bufs=6) as hpapool,
            tc.tile_pool(name="attn", bufs=4) as attnpool,
            tc.tile_pool(name="outbuf", bufs=3) as outpool,
            # PSUM budget (8 banks): shared [128,512] pool 2 + L 2x3 = 8
            tc.tile_pool(name="ps_sh", bufs=2, space="PSUM") as pssh,
            tc.tile_pool(name="ps_l", bufs=3, space="PSUM") as psl,
        ):
            # ---- constants ----
            w_allr = cpool.tile([128, HEADS * DOUT], bf16, tag="w_allr")
            nc.sync.dma_start(w_allr[:], w_allr_d[:])
            a_mats = cpool.tile([128, 128], bf16, tag="a_mats")
            nc.sync.dma_start(a_mats[:], a_mats_d[:])
            # bi tiles (one per graph parity): rows 1-8 hold the constant
            # blockind; row 0 gets the per-graph flattened src row.
            bi_ts = []
            for par in range(2):
                bi_t = cpool.tile([9, HEADS * N], f16, tag=f"bi{par}")
                nc.sync.dma_start(bi_t[1:9, :], blockind_d[:])
                bi_ts.append(bi_t)

            def psh():
                return pssh.tile([128, 512], f32, tag="ps", name="ps")

            def stage1(b0):
                """Logits-side work for pair b0: returns carried handles."""
                in_t = inpool.tile([128, 4 * N], bf16, tag="in")
                nc.sync.dma_start(
                    in_t[:], inp_d[b0 : b0 + 2].rearrange("b r c -> r b c")
                )

                # h_primeT in two [128,512] chunks; tanh each into tT
                # tT[(q*64+o), p*256 + g*128 + n] for head 2p+q
                tT_t = midpool.tile([128, 1024], bf16, tag="tT")
                for ph in range(2):
                    hpT_ps = psh()
                    for pp in range(2):
                        p = 2 * ph + pp
                        nc.tensor.matmul(
                            hpT_ps[:, pp * 256 : (pp + 1) * 256].rearrange(
                                "m (g n) -> m g n", g=2
                            ),
                            lhsT=w_allr[0:64, p * 128 : (p + 1) * 128],
                            rhs=in_t[0:64, :].rearrange("k (g c) -> k g c", g=2)[
                                :, :, 0:N
                            ],
                            start=True,
                            stop=True,
                        )
                    nc.scalar.activation(
                        tT_t[:, ph * 512 : (ph + 1) * 512], hpT_ps[:], AF.Tanh
                    )

                # src/dst coefficient rows: S[c, g*128+n], c 0-7 dst, 8-15 src
                S_ps = psh()
                for p in range(4):
                    nc.tensor.matmul(
                        S_ps[0:32, 0 : 2 * N],
                        lhsT=a_mats[:, 32 * p : 32 * (p + 1)],
                        rhs=tT_t[:, p * 256 : (p + 1) * 256],
                        start=(p == 0),
                        stop=(p == 3),
                    )
                S_sb = midpool.tile([16, 2 * N], f16, tag="S_sb")
                nc.vector.tensor_copy(S_sb[:], S_ps[0:16, 0 : 2 * N])

                # pair logits lhsT: row 0 = ones (once per ring slot),
                # rows 1-8 = dst coef rows for both graphs (gathered in
                # stage1b, after the previous pair's logits matmuls).
                lhsT9 = midpool.tile([9, 2 * N], f16, tag="lhsT9")
                if b0 < 8:  # 1 alloc/pair over a 4-deep ring
                    nc.vector.memset(lhsT9[0:1, :], 1.0)

                hpas = []
                for q in range(2):
                    bi_t = bi_ts[q]
                    qc = slice(q * N, (q + 1) * N)

                    # h_prime natural [n, h*64+o]
                    hp_ps = psh()
                    nc.tensor.matmul(
                        hp_ps[:],
                        lhsT=in_t[0:64, q * 2 * N : q * 2 * N + N],
                        rhs=w_allr[0:64, :],
                        start=True,
                        stop=True,
                    )
                    hpa_t = hpapool.tile([128, HEADS * HW], bf16, tag="hpa")
                    hpa_v = hpa_t[:].rearrange("p (h c) -> p h c", c=HW)
                    if b0 < 12:  # 2 allocs/pair over a 6-deep ring
                        nc.gpsimd.memset(hpa_v[:, :, DOUT], 1.0)
                    nc.vector.tensor_copy(
                        hpa_v[:, :, 0:DOUT],
                        hp_ps[:].rearrange("p (h c) -> p h c", c=DOUT),
                    )
                    hpas.append(hpa_t)

                return {
                    "b0": b0,
                    "in": in_t,
                    "hpas": hpas,
                    "lhsT9": lhsT9,
                    "S_sb": S_sb,
                }

            def stage1b(st):
                """Coefficient gathers + logits matmuls — emitted AFTER the
                previous pair's logits matmuls so the bi src rows and the
                3-deep L ring are never clobbered before their readers are
                in the program."""
                lhsT9, S_sb = st["lhsT9"], st["S_sb"]
                nc.sync.dma_start(lhsT9[1:9, :], S_sb[0:8, :])
                for q in range(2):
                    # src coef rows -> flattened [1, 1024] row 0 of bi
                    nc.sync.dma_start(
                        bi_ts[q][0:1, :].rearrange("p (h n) -> p h n", n=N),
                        S_sb[8 : 2 * 8, q * N : (q + 1) * N],
                    )
                Ls = []
                for q in range(2):
                    L_ps = psl.tile([128, 1024], f32, tag="L")
                    for cl in (0, 512):
                        nc.tensor.matmul(
                            L_ps[:, cl : cl + 512],
                            lhsT=lhsT9[:, q * N : (q + 1) * N],
                            rhs=bi_ts[q][:, cl : cl + 512],
                            start=True,
                            stop=True,
                        )
                    Ls.append(L_ps)
                st["Ls"] = Ls

            def stage2a(st):
                """prelu/exp/mask for the carried pair (ACT+DVE work that
                gates nothing on the PE queue)."""
                in_t, Ls = st["in"], st["Ls"]
                Ems = []
                for q in range(2):
                    L_ps = Ls[q]
                    nc.scalar.activation(L_ps[:], L_ps[:], AF.Prelu, alpha=0.2)
                    E_t = attnpool.tile([128, 1024], bf16, tag="E")
                    nc.scalar.activation(E_t[:], L_ps[:], AF.Exp)
                    Em_t = attnpool.tile([128, 1024], bf16, tag="Em")
                    adj_q = in_t[:, q * 2 * N + N : (q + 1) * 2 * N]
                    # mask-mul split: heads 0-3 on DVE, 4-7 on idle GPSIMD
                    nc.vector.tensor_mul(
                        Em_t[:, 0:512].rearrange("p (h i) -> p h i", i=N),
                        E_t[:, 0:512].rearrange("p (h i) -> p h i", i=N),
                        adj_q.unsqueeze(1).broadcast_to([N, 4, N]),
                    )
                    nc.gpsimd.tensor_mul(
                        Em_t[:, 512:1024].rearrange("p (h i) -> p h i", i=N),
                        E_t[:, 512:1024].rearrange("p (h i) -> p h i", i=N),
                        adj_q.unsqueeze(1).broadcast_to([N, 4, N]),
                    )
                    Ems.append(Em_t)
                st["Ems"] = Ems

            def stage2b(st):
                """numerator + output for the carried pair — emitted after
                the NEXT pair's logits matmuls so a stalled numerator
                (waiting on the mask) never blocks them in the PE FIFO."""
                b0, hpas, Ems = st["b0"], st["hpas"], st["Ems"]
                out_sb = outpool.tile([128, 2 * HEADS * HW], bf16, tag="out_sb")
                for q in range(2):
                    # num[i, h*65+c], col 64 = s_i
                    for half in range(2):
                        num = psh()
                        for hh in range(4):
                            h = 4 * half + hh
                            nc.tensor.matmul(
                                num[:, hh * HW : (hh + 1) * HW],
                                lhsT=Ems[q][:, h * N : (h + 1) * N],
                                rhs=hpas[q][:, h * HW : (h + 1) * HW],
                                start=True,
                                stop=True,
                            )
                        nc.vector.tensor_copy(
                            out_sb[
                                :, (2 * q + half) * 4 * HW : (2 * q + half + 1) * 4 * HW
                            ],
                            num[:, 0 : 4 * HW],
                        )
                nc.sync.dma_start(
                    out_d[b0 : b0 + 2].rearrange("b n c -> n b c"), out_sb[:]
                )

            sts = []
            for b0 in range(0, BSH, 2):
                sts.append(stage1(b0))
                if len(sts) >= 3:
                    stage2a(sts[-3])
                if len(sts) >= 2:
                    stage1b(sts[-2])
                if len(sts) >= 3:
                    stage2b(sts[-3])
            stage2a(sts[-2])
            stage1b(sts[-1])
            stage2b(sts[-2])
            stage2a(sts[-1])
            stage2b(sts[-1])

    _split_excess_waits(nc)
    return nc


def _split_excess_waits(nc, cap=1):
    """Walrus codegen accepts at most `cap` sync-wait commands per
    instruction; hoist excess waits onto standalone drains inserted before."""
    import concourse.mybir as mybir

    n_new = 0
    for _bbname, bbw in nc.bb_map.items():
        inner = bbw.bb
        il = list(inner.instructions)
        out, changed = [], False
        for inst in il:
            si = inst.sync_info
            waits = list(si.on_wait) if si and si.on_wait else []
            if len(waits) > cap:
                extra = waits[:-cap]
                for ci in range(0, len(extra), cap):
                    chunk = extra[ci : ci + cap]
                    nop = mybir.InstDrain(
                        name=f"{inst.name}_wsplit{ci}", ins=[], outs=[],
                        bass_is_fusable=False,
                    )
                    nop.engine = inst.engine
                    nop.sync_info = mybir.SyncInfo(on_wait=chunk, on_update=[])
                    nc.register_instruction(nop)
                    out.append(nop)
                    n_new += 1
                si.on_wait = waits[-cap:]
                changed = True
            out.append(inst)
        if changed:
            inner.instructions = out
    return n_new


def _host_prep(h, adj, w, a_src, a_dst):
    import ml_dtypes

    bf = ml_dtypes.bfloat16
    hT = np.ascontiguousarray(h.transpose(0, 2, 1))  # [BS, DIN, N]
    inp = np.empty((BS, 128, 2 * N), np.float32)
    inp[:, 0:DIN, 0:N] = hT
    inp[:, DIN:128, 0:N] = hT
    inp[:, :, N:] = adj.transpose(0, 2, 1)  # adjT 0/1
    inp = inp.astype(bf)
    w_all = np.ascontiguousarray(w.transpose(1, 0, 2).reshape(DIN, HEADS * DOUT))
    w_allr = np.concatenate([w_all, w_all], axis=0).astype(bf)  # [128, 512]
    # a_mats column group p (32 wide, rows (q*64+o) hold head 2p+q):
    #   local col h: a_dst[h]; 8+h: a_src[h]
    a_mats = np.zeros((128, 128), np.float32)
    for p in range(4):
        for r in range(2):
            hh = 2 * p + r
            rows = slice(r * 64, (r + 1) * 64)
            a_mats[rows, 32 * p + hh] = a_dst[hh, :, 0]
            a_mats[rows, 32 * p + 8 + hh] = a_src[hh, :, 0]
    a_mats = a_mats.astype(bf)
    blockind = np.zeros((HEADS, HEADS * N), np.float16)
    for k in range(HEADS):
        blockind[k, k * N : (k + 1) * N] = 1.0
    return inp, w_allr, a_mats, blockind


def _make_in_maps(h, adj, w, a_src, a_dst):
    inp, w_allr, a_mats, blockind = _host_prep(h, adj, w, a_src, a_dst)
    in_maps = []
    for c in range(NCORES):
        sl = slice(c * BSH, (c + 1) * BSH)
        in_maps.append(
            {
                "inp": np.ascontiguousarray(inp[sl]),
                "w_allr": w_allr,
                "a_mats": a_mats,
                "blockind": blockind,
            }
        )
    return in_maps


def _gather(results, bias):
    # results[c]["out"]: [BSH, N, HEADS*65] bf16 (num cols 0-63, s col 64)
    full = np.concatenate([results[c]["out"] for c in range(NCORES)], axis=0)
    full = full.astype(np.float32).reshape(BS, N, HEADS, DOUT + 1)
    num = full[..., :DOUT]  # [b, i, h, o]
    s = full[..., DOUT:]  # [b, i, h, 1]
    out = (num / s).transpose(0, 2, 1, 3)  # [b, h, i, o]
    return np.ascontiguousarray(out + bias[None, None, None, :]).astype(np.float32)


def kernel(h, adj, w, a_src, a_dst, bias, _trace=False):
    from concourse.bass_utils import run_bass_kernel_spmd

    h = np.asarray(h, np.float32)
    adj = np.asarray(adj, bool)
    w = np.asarray(w, np.float32)
    a_src = np.asarray(a_src, np.float32)
    a_dst = np.asarray(a_dst, np.float32)
    bias = np.asarray(bias, np.float32)

    if "nc" not in _cache:
        _cache["nc"] = _build_nc()
    nc = _cache["nc"]

    in_maps = _make_in_maps(h, adj, w, a_src, a_dst)
    res = run_bass_kernel_spmd(nc, in_maps, core_ids=list(range(NCORES)), trace=_trace)
    out = _gather(res.results, bias)
    if _trace:
        _cache["last_result"] = res
    return out


# revision 49
# speedup vs baseline: 1.0341x; 1.0341x over previous
"""Multi-head graph attention (GAT-style) Trainium2 kernel, v7.

Problem: out[b,h,i,o] = softmax_j(mask(leakyrelu_0.2(src[b,h,i] + dst[b,h,j])))
         @ h_prime[b,h,:,:] + bias
with h_prime = h @ w[h], src/dst = tanh(h_prime) @ a_src/a_dst.

Pure data-parallel over the 512-graph batch across 8 NeuronCores (64
graphs/core).  The kernel SOFTWARE-PIPELINES pairs of graphs three deep: iteration k
emits stage1(pair k) [tanh / coef rows / h_prime], stage2a(pair k-2)
[prelu / exp / mask], stage1b(pair k-1) [coef gathers + logits matmuls],
stage2b(pair k-2) [numerator / output].  Engines have strict-FIFO
queues, so this emission order keeps ACT fed (tanh of a later pair runs
while an earlier pair's logits chain is in flight) and keeps a stalled
numerator from blocking logits matmuls in the PE queue.

PSUM (8 banks): one shared [128,512] pool (bufs=2) rotates h_primeT
halves, coef rows, h_prime, and numerator tiles; logits tiles are
[128,1024] (2 banks) x bufs=3 so the pipelined emission never inverts
writer/reader order on a bank.

The unnormalized numerator + row sums ship to HBM in bf16; the softmax
division, bias add, and final transpose happen on the host.
"""

import numpy as np

BS, N, HEADS, DIN, DOUT = 512, 128, 8, 64, 64
NCORES = 8
BSH = BS // NCORES  # graphs per core

_cache = {}


def _build_nc():
    import concourse.bass as bass
    import concourse.mybir as mybir
    import concourse.tile as tile

    f32 = mybir.dt.float32
    f16 = mybir.dt.float16
    bf16 = mybir.dt.bfloat16
    AF = mybir.ActivationFunctionType
    HW = DOUT + 1  # 65: per-head numerator cols + row-sum column

    nc = bass.Bass("TRN2", target_bir_lowering=False, debug=False)

    # inp[b] = [hTr | adjT] side by side: cols 0-127 hT (DIN rows doubled),
    # cols 128-255 adjT (0/1).
    inp_d = nc.dram_tensor("inp", [BSH, 128, 2 * N], bf16, kind="ExternalInput").ap()
    w_allr_d = nc.dram_tensor("w_allr", [128, HEADS * DOUT], bf16, kind="ExternalInput").ap()
    a_mats_d = nc.dram_tensor("a_mats", [128, 128], bf16, kind="ExternalInput").ap()
    blockind_d = nc.dram_tensor("blockind", [HEADS, HEADS * N], f16, kind="ExternalInput").ap()
    out_d = nc.dram_tensor("out", [BSH, N, HEADS * HW], bf16, kind="ExternalOutput").ap()

    with tile.TileContext(nc) as tc:
        with (
            tc.tile_pool(name="consts", bufs=1) as cpool,
            tc.tile_pool(name="inbuf", bufs=4) as inpool,
            tc.tile_pool(name="mid", bufs=5) as midpool,
            tc.tile_pool(name="hpabuf", bufs=8) as hpapool,
            tc.tile_pool(name="attn", bufs=5) as attnpool,
            tc.tile_pool(name="outbuf", bufs=4) as outpool,
            # PSUM budget (8 banks): shared [128,512] pool 2 + L 2x3 = 8
            tc.tile_pool(name="ps_sh", bufs=2, space="PSUM") as pssh,
            tc.tile_pool(name="ps_l", bufs=3, space="PSUM") as psl,
        ):
            # ---- constants ----
            w_allr = cpool.tile([128, HEADS * DOUT], bf16, tag="w_allr")
            nc.sync.dma_start(w_allr[:], w_allr_d[:])
            a_mats = cpool.tile([128, 128], bf16, tag="a_mats")
            nc.sync.dma_start(a_mats[:], a_mats_d[:])
            # bi tiles (one per graph parity): rows 1-8 hold the constant
            # blockind; row 0 gets the per-graph flattened src row.
            bi_ts = []
            for par in range(2):
                bi_t = cpool.tile([9, HEADS * N], f16, tag=f"bi{par}")
                nc.sync.dma_start(bi_t[1:9, :], blockind_d[:])
                bi_ts.append(bi_t)

            def psh():
                return pssh.tile([128, 512], f32, tag="ps", name="ps")

            def stage1(b0):
                """Logits-side work for pair b0: returns carried handles."""
                in_t = inpool.tile([128, 4 * N], bf16, tag="in")
                nc.sync.dma_start(
                    in_t[:], inp_d[b0 : b0 + 2].rearrange("b r c -> r b c")
                )

                # h_primeT in two [128,512] chunks; tanh each into tT
                # tT[(q*64+o), p*256 + g*128 + n] for head 2p+q
                tT_t = midpool.tile([128, 1024], bf16, tag="tT")
                for ph in range(2):
                    hpT_ps = psh()
                    for pp in range(2):
                        p = 2 * ph + pp
                        nc.tensor.matmul(
                            hpT_ps[:, pp * 256 : (pp + 1) * 256].rearrange(
                                "m (g n) -> m g n", g=2
                            ),
                            lhsT=w_allr[0:64, p * 128 : (p + 1) * 128],
                            rhs=in_t[0:64, :].rearrange("k (g c) -> k g c", g=2)[
                                :, :, 0:N
                            ],
                            start=True,
                            stop=True,
                        )
                    nc.scalar.activation(
                        tT_t[:, ph * 512 : (ph + 1) * 512], hpT_ps[:], AF.Tanh
                    )

                # src/dst coefficient rows: S[c, g*128+n], c 0-7 dst, 8-15 src
                S_ps = psh()
                for p in range(4):
                    nc.tensor.matmul(
                        S_ps[0:32, 0 : 2 * N],
                        lhsT=a_mats[:, 32 * p : 32 * (p + 1)],
                        rhs=tT_t[:, p * 256 : (p + 1) * 256],
                        start=(p == 0),
                        stop=(p == 3),
                    )
                S_sb = midpool.tile([16, 2 * N], f16, tag="S_sb")
                nc.vector.tensor_copy(S_sb[:], S_ps[0:16, 0 : 2 * N])

                # pair logits lhsT: row 0 = ones (once per ring slot),
                # rows 1-8 = dst coef rows for both graphs (gathered in
                # stage1b, after the previous pair's logits matmuls).
                lhsT9 = midpool.tile([9, 2 * N], f16, tag="lhsT9")
                if b0 < 10:  # 1 alloc/pair over a 5-deep ring
                    nc.vector.memset(lhsT9[0:1, :], 1.0)

                hpas = []
                for q in range(2):
                    bi_t = bi_ts[q]
                    qc = slice(q * N, (q + 1) * N)

                    # h_prime natural [n, h*64+o]
                    hp_ps = psh()
                    nc.tensor.matmul(
                        hp_ps[:],
                        lhsT=in_t[0:64, q * 2 * N : q * 2 * N + N],
                        rhs=w_allr[0:64, :],
                        start=True,
                        stop=True,
                    )
                    hpa_t = hpapool.tile([128, HEADS * HW], bf16, tag="hpa")
                    hpa_v = hpa_t[:].rearrange("p (h c) -> p h c", c=HW)
                    if b0 < 16:  # 2 allocs/pair over an 8-deep ring
                        nc.gpsimd.memset(hpa_v[:, :, DOUT], 1.0)
                    nc.vector.tensor_copy(
                        hpa_v[:, :, 0:DOUT],
                        hp_ps[:].rearrange("p (h c) -> p h c", c=DOUT),
                    )
                    hpas.append(hpa_t)

                return {
                    "b0": b0,
                    "in": in_t,
                    "hpas": hpas,
                    "lhsT9": lhsT9,
                    "S_sb": S_sb,
                }

            def stage1b(st):
                """Coefficient gathers + logits matmuls — emitted AFTER the
                previous pair's logits matmuls so the bi src rows and the
                3-deep L ring are never clobbered before their readers are
                in the program."""
                lhsT9, S_sb = st["lhsT9"], st["S_sb"]
                nc.sync.dma_start(lhsT9[1:9, :], S_sb[0:8, :])
                for q in range(2):
                    # src coef rows -> flattened [1, 1024] row 0 of bi
                    nc.sync.dma_start(
                        bi_ts[q][0:1, :].rearrange("p (h n) -> p h n", n=N),
                        S_sb[8 : 2 * 8, q * N : (q + 1) * N],
                    )
                Ls = []
                for q in range(2):
                    L_ps = psl.tile([128, 1024], f32, tag="L")
                    for cl in (0, 512):
                        nc.tensor.matmul(
                            L_ps[:, cl : cl + 512],
                            lhsT=lhsT9[:, q * N : (q + 1) * N],
                            rhs=bi_ts[q][:, cl : cl + 512],
                            start=True,
                            stop=True,
                        )
                    Ls.append(L_ps)
                st["Ls"] = Ls

            def stage2a(st):
                """prelu/exp/mask for the carried pair (ACT+DVE work that
                gates nothing on the PE queue)."""
                in_t, Ls = st["in"], st["Ls"]
                Ems = []
                for q in range(2):
                    L_ps = Ls[q]
                    nc.scalar.activation(L_ps[:], L_ps[:], AF.Prelu, alpha=0.2)
                    E_t = attnpool.tile([128, 1024], bf16, tag="E")
                    nc.scalar.activation(E_t[:], L_ps[:], AF.Exp)
                    Em_t = attnpool.tile([128, 1024], bf16, tag="Em")
                    adj_q = in_t[:, q * 2 * N + N : (q + 1) * 2 * N]
                    nc.vector.tensor_mul(
                        Em_t[:].rearrange("p (h i) -> p h i", i=N),
                        E_t[:].rearrange("p (h i) -> p h i", i=N),
                        adj_q.unsqueeze(1).broadcast_to([N, HEADS, N]),
                    )
                    Ems.append(Em_t)
                st["Ems"] = Ems

            def stage2b(st):
                """numerator + output for the carried pair — emitted after
                the NEXT pair's logits matmuls so a stalled numerator
                (waiting on the mask) never blocks them in the PE FIFO."""
                b0, hpas, Ems = st["b0"], st["hpas"], st["Ems"]
                out_sb = outpool.tile([128, 2 * HEADS * HW], bf16, tag="out_sb")
                for q in range(2):
                    # num[i, h*65+c], col 64 = s_i
                    for half in range(2):
                        num = psh()
                        for hh in range(4):
                            h = 4 * half + hh
                            nc.tensor.matmul(
                                num[:, hh * HW : (hh + 1) * HW],
                                lhsT=Ems[q][:, h * N : (h + 1) * N],
                                rhs=hpas[q][:, h * HW : (h + 1) * HW],
                                start=True,
                                stop=True,
                            )
                        nc.vector.tensor_copy(
                            out_sb[
                                :, (2 * q + half) * 4 * HW : (2 * q + half + 1) * 4 * HW
                            ],
                            num[:, 0 : 4 * HW],
                        )
                nc.sync.dma_start(
                    out_d[b0 : b0 + 2].rearrange("b n c -> n b c"), out_sb[:]
                )

            sts = []
            for b0 in range(0, BSH, 2):
                sts.append(stage1(b0))
                if len(sts) >= 3:
                    stage2a(sts[-3])
                if len(sts) >= 2:
                    stage1b(sts[-2])
                if len(sts) >= 3:
                    stage2b(sts[-3])
            stage2a(sts[-2])
            stage1b(sts[-1])
            stage2b(sts[-2])
            stage2a(sts[-1])
            stage2b(sts[-1])

    _split_excess_waits(nc)
    return nc


def _split_excess_waits(nc, cap=1):
    """Walrus codegen accepts at most `cap` sync-wait commands per
    instruction; hoist excess waits onto standalone drains inserted before."""
    import concourse.mybir as mybir

    n_new = 0
    for _bbname, bbw in nc.bb_map.items():
        inner = bbw.bb
        il = list(inner.instructions)
        out, changed = [], False
        for inst in il:
            si = inst.sync_info
            waits = list(si.on_wait) if si and si.on_wait else []
            if len(waits) > cap:
                extra = waits[:-cap]
                for ci in range(0, len(extra), cap):
                    chunk = extra[ci : ci + cap]
                    nop = mybir.InstDrain(
                        name=f"{inst.name}_wsplit{ci}", ins=[], outs=[],
                        bass_is_fusable=False,
                    )
                    nop.engine = inst.engine
                    nop.sync_info = mybir.SyncInfo(on_wait=chunk, on_update=[])
                    nc.register_instruction(nop)
                    out.append(nop)
                    n_new += 1
                si.on_wait = waits[-cap:]
                changed = True
            out.append(inst)
        if changed:
            inner.instructions = out
    return n_new


def _host_prep(h, adj, w, a_src, a_dst):
    import ml_dtypes

    bf = ml_dtypes.bfloat16
    hT = np.ascontiguousarray(h.transpose(0, 2, 1))  # [BS, DIN, N]
    inp = np.empty((BS, 128, 2 * N), np.float32)
    inp[:, 0:DIN, 0:N] = hT
    inp[:, DIN:128, 0:N] = hT
    inp[:, :, N:] = adj.transpose(0, 2, 1)  # adjT 0/1
    inp = inp.astype(bf)
    w_all = np.ascontiguousarray(w.transpose(1, 0, 2).reshape(DIN, HEADS * DOUT))
    w_allr = np.concatenate([w_all, w_all], axis=0).astype(bf)  # [128, 512]
    # a_mats column group p (32 wide, rows (q*64+o) hold head 2p+q):
    #   local col h: a_dst[h]; 8+h: a_src[h]
    a_mats = np.zeros((128, 128), np.float32)
    for p in range(4):
        for r in range(2):
            hh = 2 * p + r
            rows = slice(r * 64, (r + 1) * 64)
            a_mats[rows, 32 * p + hh] = a_dst[hh, :, 0]
            a_mats[rows, 32 * p + 8 + hh] = a_src[hh, :, 0]
    a_mats = a_mats.astype(bf)
    blockind = np.zeros((HEADS, HEADS * N), np.float16)
    for k in range(HEADS):
        blockind[k, k * N : (k + 1) * N] = 1.0
    return inp, w_allr, a_mats, blockind


def _make_in_maps(h, adj, w, a_src, a_dst):
    inp, w_allr, a_mats, blockind = _host_prep(h, adj, w, a_src, a_dst)
    in_maps = []
    for c in range(NCORES):
        sl = slice(c * BSH, (c + 1) * BSH)
        in_maps.append(
            {
                "inp": np.ascontiguousarray(inp[sl]),
                "w_allr": w_allr,
                "a_mats": a_mats,
                "blockind": blockind,
            }
        )
    return in_maps


def _gather(results, bias):
    # results[c]["out"]: [BSH, N, HEADS*65] bf16 (num cols 0-63, s col 64)
    full = np.concatenate([results[c]["out"] for c in range(NCORES)], axis=0)
    full = full.astype(np.float32).reshape(BS, N, HEADS, DOUT + 1)
    num = full[..., :DOUT]  # [b, i, h, o]
    s = full[..., DOUT:]  # [b, i, h, 1]
    out = (num / s).transpose(0, 2, 1, 3)  # [b, h, i, o]
    return np.ascontiguousarray(out + bias[None, None, None, :]).astype(np.float32)


def kernel(h, adj, w, a_src, a_dst, bias, _trace=False):
    from concourse.bass_utils import run_bass_kernel_spmd

    h = np.asarray(h, np.float32)
    adj = np.asarray(adj, bool)
    w = np.asarray(w, np.float32)
    a_src = np.asarray(a_src, np.float32)
    a_dst = np.asarray(a_dst, np.float32)
    bias = np.asarray(bias, np.float32)

    if "nc" not in _cache:
        _cache["nc"] = _build_nc()
    nc = _cache["nc"]

    in_maps = _make_in_maps(h, adj, w, a_src, a_dst)
    res = run_bass_kernel_spmd(nc, in_maps, core_ids=list(range(NCORES)), trace=_trace)
    out = _gather(res.results, bias)
    if _trace:
        _cache["last_result"] = res
    return out
